# revision 27
# baseline (speedup 1.0000x reference)
"""LDA loss (inter/intra hinge) on 8 Trainium2 NeuronCores — fused launch.

Per core (data-parallel over B, hence over G = B/16 centers):

  Host glue: centers c = group means (O(G*D) numpy), sq = ||c||^2, fp8
    (e4m3) quantized centers rotated so each core's own 1024-center
    block leads, followed by the next 4 blocks (this symmetric schedule
    covers every unordered pair at least once).  Per-row certification
    thresholds T_i = (sq_i + min_sq - 1 - 2*eg_i)/2 where eg_i is a
    rigorous bound on the fp8 gram error derived from the exact
    quantization residuals.

  Device, one launch:
    - inter detector: psum = C_loc^T C_cols via fp8 matmuls; the own
      (diagonal) block only computes columns j >= 128*m (upper
      triangle), with -384*I added on the diagonal itself.  Each psum
      tile is consumed either by ACT relu(psum - T_i) with accumulation
      (violation mass, exactly 0 when all grams are certified-far) or by
      DVE max-reduce (per-row max gram); the split is load-balanced.
    - intra: diff = (I - P) @ x via f32r matmuls (P = group-mean
      selector); DVE bn_stats on the PSUM diff gives per-sample
      sum(diff^2) = s2 + s5 + 64*(s1^2 + s4^2); an exact hinge tail
      computes sum(max(d - 0.1, 0)^2) -> ipart.  Feature chunks are
      scheduled late (after their DMA lands) so the in-order PE never
      stalls the detector stream.

  Host check: rows with relu mass > 0 or max-gram above T_i are
    "suspect"; suspect rows are re-verified exactly in fp64 (never taken
    for in-margin data).  If every pair is certified > margin, every
    hinge term is exactly 0 and inter == 0.0 bit-equal to the reference;
    otherwise fall back to the exact fp64 computation.

  intra = sum(ipart) / B.
"""
import sys

if "/opt/trn_rl_repo" not in sys.path:
    sys.path.insert(0, "/opt/trn_rl_repo")

import numpy as np
import ml_dtypes

import concourse.bacc as bacc
import concourse.tile as tile
from concourse import mybir
from concourse.bass_utils import run_bass_kernel_spmd

N_CORES = 8
B, D, P = 131072, 128, 16
G = B // P                 # 8192 centers
GL = G // N_CORES          # 1024 local centers
SL = B // N_CORES          # 16384 local samples
NBLK = 5                   # column blocks per core in the detector
NC2 = NBLK * GL            # 5120 detector columns
NSLOT = NBLK * 8           # detector (m, t) slots
BIG = 384.0                # e4m3-exact diagonal suppressor
MARGIN_INTRA = 0.1
MARGIN_INTER = 1.0
# intra feature tiles processed in detector row-chunk m (DMA-arrival paced)
INTRA_SCHED = [0, 0, 2, 3, 3, 4, 4, 0]

F32 = mybir.dt.float32
F32R = mybir.dt.float32r
BF16 = mybir.dt.bfloat16
FP8 = mybir.dt.float8e4
AF = mybir.ActivationFunctionType

_cache = {}
_last_traces = {}


def _det_plan():
    """Static (m, t) -> engine assignment, load-balanced greedily.
    Returns set of slots handled by ACT (rest go to DVE max-reduce)."""
    act_slots = set()
    load_a = 26200.0   # ACT: 32 intra square evictions + hinge tail + accum reads
    load_d = 20300.0   # DVE: 16 paired intra reduces + memsets
    for m in range(8):
        for t in range(NBLK):
            w = 1024 - 128 * m if t == 0 else 1024
            ca = 356 + w * 0.833 + 286    # measured ACT relu+accum cost
            cd = 80 + w * 1.042 + 45      # measured DVE max-reduce cost
            if load_a + ca <= load_d + cd:
                act_slots.add(NBLK * m + t)
                load_a += ca
            else:
                load_d += cd
    return act_slots


ACT_SLOTS = _det_plan()


def _build_fused():
    nc = bacc.Bacc("TRN2", target_bir_lowering=False, debug=False,
                   num_devices=N_CORES)
    fea = nc.dram_tensor("fea", [SL, D], F32R, kind="ExternalInput").ap()
    m128 = nc.dram_tensor("m128", [128, 128], F32R, kind="ExternalInput").ap()
    ctr8 = nc.dram_tensor("ctr8", [128, NC2], FP8, kind="ExternalInput").ap()
    negT = nc.dram_tensor("negT", [128, 8], F32, kind="ExternalInput").ap()
    nbig = nc.dram_tensor("nbig", [128, 128], FP8, kind="ExternalInput").ap()
    idI = nc.dram_tensor("idI", [128, 128], FP8, kind="ExternalInput").ap()
    vm = nc.dram_tensor("vm", [128, NSLOT], F32, kind="ExternalOutput").ap()
    mg = nc.dram_tensor("mg", [128, NSLOT], F32, kind="ExternalOutput").ap()
    ipart = nc.dram_tensor("ipart", [128, 1], F32, kind="ExternalOutput").ap()

    fea3 = fea.rearrange("(b p) d -> p b d", p=128)  # [128, 128, 128]

    with tile.TileContext(nc) as tc:
        with (
            tc.tile_pool(name="fea", bufs=1) as fp,
            tc.tile_pool(name="small", bufs=1) as sp,
            tc.tile_pool(name="dum", bufs=2) as dp,
            tc.tile_pool(name="sqd", bufs=2) as sqp,
            tc.tile_pool(name="psD", bufs=3, space="PSUM") as psd,
            tc.tile_pool(name="psA", bufs=2, space="PSUM") as psa,
        ):
            # detector columns first: PE starts as soon as block 0 lands
            t_c8 = []
            for k in range(NBLK):
                t = sp.tile([128, 1024], FP8, tag=f"c8{k}")
                eng = nc.scalar if k < 2 else nc.sync
                eng.dma_start(t[:], ctr8[:, 1024 * k:1024 * (k + 1)])
                t_c8.append(t)
            t_nT = sp.tile([128, 8], F32, tag="nT")
            nc.sync.dma_start(t_nT[:], negT[:])
            t_nb = sp.tile([128, 128], FP8, tag="nb")
            nc.sync.dma_start(t_nb[:], nbig[:])
            t_id = sp.tile([128, 128], FP8, tag="id")
            nc.sync.dma_start(t_id[:], idI[:])
            t_m = sp.tile([128, 128], F32R, tag="m")
            nc.sync.dma_start(t_m[:], m128[:])
            t_fs = []
            for k in range(16):
                t = fp.tile([128, 1024], F32R, tag=f"fea{k}")
                nc.sync.dma_start(
                    t[:].rearrange("p (b d) -> p b d", d=128),
                    fea3[:, 8 * k:8 * (k + 1), :])
                t_fs.append(t)
            # pull the sqrt_and_others ACT table load off the critical tail
            t_tb = sp.tile([128, 1], F32, tag="tb")
            nc.scalar.activation(t_tb[:], t_nT[:, 0:1], AF.Sqrt,
                                 bias=0.0, scale=0.0)

            t_vm = sp.tile([128, NSLOT], F32, tag="vm")
            t_mg = sp.tile([128, NSLOT], F32, tag="mg")
            nc.vector.memset(t_vm[:], 0.0)
            nc.vector.memset(t_mg[:], 0.0)
            t_d2 = sp.tile([128, 128], F32, tag="d2")

            kin = 0   # next intra feature tile
            for m in range(8):
                lhs = t_c8[0][:, 128 * m:128 * (m + 1)]
                # ---- detector row-chunk m ----
                for t in range(NBLK):
                    ps = psd.tile([128, 1024], F32, tag="psD")
                    lo = 128 * m if t == 0 else 0
                    for h in range(2):
                        a, b = max(lo, 512 * h), 512 * (h + 1)
                        if a >= b:
                            continue
                        nc.tensor.matmul(ps[:, a:b], lhs, t_c8[t][:, a:b],
                                         start=True, stop=True)
                    if t == 0:
                        nc.tensor.matmul(ps[:, 128 * m:128 * (m + 1)],
                                         t_nb[:, :], t_id[:, :],
                                         start=False, stop=True,
                                         skip_group_check=True)
                    slot = NBLK * m + t
                    if slot in ACT_SLOTS:
                        dum = dp.tile([128, 1024], BF16, tag="dum")
                        nc.scalar.activation(dum[:, lo:1024], ps[:, lo:1024],
                                             AF.Relu, bias=t_nT[:, m:m + 1],
                                             scale=1.0,
                                             accum_out=t_vm[:, slot:slot + 1])
                    else:
                        nc.vector.tensor_reduce(t_mg[:, slot:slot + 1],
                                                ps[:, lo:1024],
                                                axis=mybir.AxisListType.X,
                                                op=mybir.AluOpType.max)
                # ---- intra feature tiles scheduled for this m ----
                for _ in range(INTRA_SCHED[m]):
                    k = kin
                    kin += 1
                    ft = t_fs[k]
                    sqd = sqp.tile([128, 1024], F32, tag="sqd")
                    for h in range(2):
                        psq = psa.tile([128, 512], F32, tag="psA")
                        nc.tensor.matmul(psq[:], t_m[:, :],
                                         ft[:, 512 * h:512 * (h + 1)],
                                         start=True, stop=True)
                        nc.scalar.activation(sqd[:, 512 * h:512 * (h + 1)],
                                             psq[:], AF.Square)
                    nc.vector.tensor_reduce(
                        t_d2[:, 8 * k:8 * (k + 1)],
                        sqd[:].rearrange("p (t d) -> p t d", d=128),
                        axis=mybir.AxisListType.X, op=mybir.AluOpType.add)
                if m == 6:
                    # first 6 row-chunks' detector slots export under m=7
                    nc.sync.dma_start(vm[:, 0:NBLK * 6], t_vm[:, 0:NBLK * 6])
                    nc.sync.dma_start(mg[:, 0:NBLK * 6], t_mg[:, 0:NBLK * 6])
                    # intra done: hinge tail + ipart export hide under m=7
                    t_d = sp.tile([128, 128], F32, tag="d")
                    nc.scalar.activation(t_d[:], t_d2[:], AF.Sqrt)
                    t_w = sp.tile([128, 128], F32, tag="w")
                    nc.vector.tensor_scalar(t_w[:], t_d[:], MARGIN_INTRA, 0.0,
                                            op0=mybir.AluOpType.subtract,
                                            op1=mybir.AluOpType.max)
                    t_w2 = sp.tile([128, 128], F32, tag="w2")
                    t_acc = sp.tile([128, 1], F32, tag="acc")
                    nc.scalar.activation(t_w2[:], t_w[:], AF.Square,
                                         accum_out=t_acc[:])
                    nc.gpsimd.dma_start(ipart[:], t_acc[:])

            nc.scalar.dma_start(vm[:, NBLK * 6:], t_vm[:, NBLK * 6:])
            nc.sync.dma_start(mg[:, NBLK * 6:], t_mg[:, NBLK * 6:])
    nc.compile()
    return nc


def _get(name, builder):
    if name not in _cache:
        _cache[name] = builder()
    return _cache[name]


def _exact_inter_host(centers):
    """Exact fp64 inter loss (full fallback)."""
    c = centers.astype(np.float64)
    sq = (c * c).sum(1)
    tot = 0.0
    for i0 in range(0, G, 1024):
        blk = sq[i0:i0 + 1024, None] + sq[None, :] - 2.0 * (c[i0:i0 + 1024] @ c.T)
        d = np.sqrt(np.maximum(blk, 0.0))
        h = np.maximum(MARGIN_INTER - d, 0.0) ** 2
        iu = np.triu(np.ones((1024, G), dtype=bool), k=1 + i0)
        tot += h[iu].sum()
    return np.float32(tot / (G * (G - 1) / 2.0))


def kernel(path_fea):
    fea = np.ascontiguousarray(
        np.asarray(path_fea, dtype=np.float32).reshape(B, D))

    trace = bool(int(__import__("os").environ.get("KERNEL_TRACE", "0")))
    runkw = {}
    if trace:
        try:
            import trace_shim
            trace_shim.install()
            runkw = dict(trace=True)
        except ImportError:
            trace = False

    # ---------------- host glue: centers, fp8 layout, thresholds --------
    centers = fea.reshape(G, P, D).mean(axis=1)              # [G, D] f32
    sq = (centers.astype(np.float64) ** 2).sum(1)
    minsq = sq.min()
    c8 = centers.astype(ml_dtypes.float8_e4m3fn)
    c8f = c8.astype(np.float64)
    delta = centers.astype(np.float64) - c8f
    dn = np.sqrt((delta ** 2).sum(1))                        # ||delta_i||
    cn = np.maximum(np.sqrt(sq), np.sqrt((c8f ** 2).sum(1)))
    # rigorous per-row bound on |gram - fp8 gram| (+ f32 accumulation slack)
    eg = dn * cn.max() + dn.max() * cn + 0.01
    T = ((sq + minsq - MARGIN_INTER - 2.0 * eg) / 2.0).astype(np.float32)

    Pm = np.zeros((128, 128), np.float32)
    for s in range(128):
        Pm[s, (s // 16) * 16:(s // 16 + 1) * 16] = 1.0 / 16.0
    m128 = np.eye(128, dtype=np.float32) - Pm

    nbig = (-BIG * np.eye(128)).astype(ml_dtypes.float8_e4m3fn)
    idI = np.eye(128, dtype=np.float32).astype(ml_dtypes.float8_e4m3fn)

    ins = []
    for c in range(N_CORES):
        idx = (np.arange(NC2) + GL * c) % G
        ctr8c = np.ascontiguousarray(c8[idx].T)              # [128, NC2]
        negT = np.ascontiguousarray(
            -T[GL * c:GL * (c + 1)].reshape(8, 128).T)       # [128, 8]
        ins.append({"fea": fea[SL * c:SL * (c + 1)], "m128": m128,
                    "ctr8": ctr8c, "negT": negT, "nbig": nbig, "idI": idI})

    ncf = _get("fused", _build_fused)
    r = run_bass_kernel_spmd(ncf, ins, core_ids=list(range(N_CORES)), **runkw)
    if trace and r.exec_time_ns is not None:
        print(f"[fused] HW exec time: {r.exec_time_ns} ns")
        _last_traces["fused"] = r

    # ---------------- host reduction + certification ----------------
    ipart_sum = 0.0
    suspects = []
    finite = np.isfinite(T).all()
    for c in range(N_CORES):
        ipart_sum += float(r.results[c]["ipart"].astype(np.float64).sum())
        vmc = r.results[c]["vm"]
        mgc = r.results[c]["mg"]
        if not (finite and np.isfinite(vmc).all() and np.isfinite(mgc).all()):
            suspects.extend(range(GL * c, GL * (c + 1)))
            continue
        Tc = T[GL * c:GL * (c + 1)].reshape(8, 128).T        # [128, 8]
        for m in range(8):
            slots_a = [NBLK * m + t for t in range(NBLK)
                       if NBLK * m + t in ACT_SLOTS]
            slots_d = [NBLK * m + t for t in range(NBLK)
                       if NBLK * m + t not in ACT_SLOTS]
            bad = np.zeros(128, bool)
            if slots_a:
                bad |= (vmc[:, slots_a].sum(axis=1) > 0.0)
            if slots_d:
                bad |= (mgc[:, slots_d].max(axis=1) > Tc[:, m])
            for p in np.nonzero(bad)[0]:
                suspects.append(GL * c + 128 * m + int(p))
    intra = np.float32(ipart_sum / B)

    inter = np.float32(0.0)
    if suspects:
        # exact fp64 recheck of suspect rows against all centers
        cd = centers.astype(np.float64)
        sqd_ = (cd * cd).sum(1)
        ok = True
        for i in suspects:
            d2 = sqd_[i] + sqd_ - 2.0 * (cd @ cd[i])
            d2[i] = np.inf
            if d2.min() <= MARGIN_INTER ** 2:
                ok = False
                break
        if not ok:
            inter = _exact_inter_host(centers)
    return (inter, intra)


# revision 28
# speedup vs baseline: 1.0726x; 1.0726x over previous
"""LDA loss (inter/intra hinge) on 8 Trainium2 NeuronCores — fused launch.

Per core (data-parallel over B, hence over G = B/16 centers):

  Host glue: centers c = group means (O(G*D) numpy), sq = ||c||^2, fp8
    (e4m3) quantized centers rotated so each core's own 1024-center
    block leads, followed by the next 4 blocks (this symmetric schedule
    covers every unordered pair at least once).  Per-row certification
    thresholds T_i = (sq_i + min_sq - 1 - 2*eg_i)/2 where eg_i is a
    rigorous bound on the fp8 gram error derived from the exact
    quantization residuals.

  Device, one launch:
    - inter detector: psum = C_loc^T C_cols via fp8 matmuls; the own
      (diagonal) block only computes columns j >= 128*m (upper
      triangle), with -384*I added on the diagonal itself.  Each psum
      tile is consumed either by ACT relu(psum - T_i) with accumulation
      (violation mass, exactly 0 when all grams are certified-far) or by
      DVE max-reduce (per-row max gram); the split is load-balanced.
    - intra: diff = (I - P) @ x via f32r matmuls (P = group-mean
      selector); DVE bn_stats on the PSUM diff gives per-sample
      sum(diff^2) = s2 + s5 + 64*(s1^2 + s4^2); an exact hinge tail
      computes sum(max(d - 0.1, 0)^2) -> ipart.  Feature chunks are
      scheduled late (after their DMA lands) so the in-order PE never
      stalls the detector stream.

  Host check: rows with relu mass > 0 or max-gram above T_i are
    "suspect"; suspect rows are re-verified exactly in fp64 (never taken
    for in-margin data).  If every pair is certified > margin, every
    hinge term is exactly 0 and inter == 0.0 bit-equal to the reference;
    otherwise fall back to the exact fp64 computation.

  intra = sum(ipart) / B.
"""
import sys

if "/opt/trn_rl_repo" not in sys.path:
    sys.path.insert(0, "/opt/trn_rl_repo")

import numpy as np
import ml_dtypes

import concourse.bacc as bacc
import concourse.tile as tile
from concourse import mybir
from concourse.bass_utils import run_bass_kernel_spmd

N_CORES = 8
B, D, P = 131072, 128, 16
G = B // P                 # 8192 centers
GL = G // N_CORES          # 1024 local centers
SL = B // N_CORES          # 16384 local samples
NBLK = 5                   # column blocks per core in the detector
NC2 = NBLK * GL            # 5120 detector columns
NSLOT = NBLK * 8           # detector (m, t) slots
BIG = 384.0                # e4m3-exact diagonal suppressor
MARGIN_INTRA = 0.1
MARGIN_INTER = 1.0
# intra feature tiles processed in detector row-chunk m (DMA-arrival paced)
INTRA_SCHED = [0, 0, 2, 3, 3, 4, 4, 0]

F32 = mybir.dt.float32
F32R = mybir.dt.float32r
BF16 = mybir.dt.bfloat16
FP8 = mybir.dt.float8e4
AF = mybir.ActivationFunctionType

_cache = {}
_last_traces = {}


def _det_plan():
    """Static (m, t) -> engine assignment, load-balanced greedily.
    Returns set of slots handled by ACT (rest go to DVE max-reduce)."""
    act_slots = set()
    load_a = 23500.0   # ACT: 32 intra square evictions + hinge tail + accum reads
    load_d = 22000.0   # DVE: 32 intra reduces + memsets
    for m in range(8):
        for t in range(NBLK):
            w = 1024 - 128 * m if t == 0 else 1024
            ca = 356 + w * 0.833 + 286    # measured ACT relu+accum cost
            cd = 80 + w * 1.042 + 45      # measured DVE max-reduce cost
            if load_a + ca <= load_d + cd:
                act_slots.add(NBLK * m + t)
                load_a += ca
            else:
                load_d += cd
    return act_slots


ACT_SLOTS = _det_plan()


def _build_fused():
    nc = bacc.Bacc("TRN2", target_bir_lowering=False, debug=False,
                   num_devices=N_CORES)
    fea = nc.dram_tensor("fea", [SL, D], F32R, kind="ExternalInput").ap()
    m128 = nc.dram_tensor("m128", [128, 128], F32R, kind="ExternalInput").ap()
    ctr8 = nc.dram_tensor("ctr8", [128, NC2], FP8, kind="ExternalInput").ap()
    negT = nc.dram_tensor("negT", [128, 8], F32, kind="ExternalInput").ap()
    nbig = nc.dram_tensor("nbig", [128, 128], FP8, kind="ExternalInput").ap()
    idI = nc.dram_tensor("idI", [128, 128], FP8, kind="ExternalInput").ap()
    vm = nc.dram_tensor("vm", [128, NSLOT], F32, kind="ExternalOutput").ap()
    mg = nc.dram_tensor("mg", [128, NSLOT], F32, kind="ExternalOutput").ap()
    ipart = nc.dram_tensor("ipart", [128, 1], F32, kind="ExternalOutput").ap()

    fea3 = fea.rearrange("(b p) d -> p b d", p=128)  # [128, 128, 128]

    with tile.TileContext(nc) as tc:
        with (
            tc.tile_pool(name="fea", bufs=1) as fp,
            tc.tile_pool(name="small", bufs=1) as sp,
            tc.tile_pool(name="dum", bufs=2) as dp,
            tc.tile_pool(name="sqd", bufs=2) as sqp,
            tc.tile_pool(name="psD", bufs=3, space="PSUM") as psd,
            tc.tile_pool(name="psA", bufs=2, space="PSUM") as psa,
        ):
            # detector columns first: PE starts as soon as block 0 lands
            t_c8 = []
            for k in range(NBLK):
                t = sp.tile([128, 1024], FP8, tag=f"c8{k}")
                eng = nc.scalar if k < 2 else nc.sync
                eng.dma_start(t[:], ctr8[:, 1024 * k:1024 * (k + 1)])
                t_c8.append(t)
            t_nT = sp.tile([128, 8], F32, tag="nT")
            nc.sync.dma_start(t_nT[:], negT[:])
            t_nb = sp.tile([128, 128], FP8, tag="nb")
            nc.sync.dma_start(t_nb[:], nbig[:])
            t_id = sp.tile([128, 128], FP8, tag="id")
            nc.sync.dma_start(t_id[:], idI[:])
            t_m = sp.tile([128, 128], F32R, tag="m")
            nc.sync.dma_start(t_m[:], m128[:])
            t_fs = []
            for k in range(16):
                t = fp.tile([128, 1024], F32R, tag=f"fea{k}")
                nc.sync.dma_start(
                    t[:].rearrange("p (b d) -> p b d", d=128),
                    fea3[:, 8 * k:8 * (k + 1), :])
                t_fs.append(t)
            # pull the sqrt_and_others ACT table load off the critical tail
            t_tb = sp.tile([128, 1], F32, tag="tb")
            nc.scalar.activation(t_tb[:], t_nT[:, 0:1], AF.Sqrt,
                                 bias=0.0, scale=0.0)

            t_vm = sp.tile([128, NSLOT], F32, tag="vm")
            t_mg = sp.tile([128, NSLOT], F32, tag="mg")
            nc.vector.memset(t_vm[:], 0.0)
            nc.vector.memset(t_mg[:], 0.0)
            t_d2 = sp.tile([128, 128], F32, tag="d2")

            kin = 0   # next intra feature tile
            for m in range(8):
                lhs = t_c8[0][:, 128 * m:128 * (m + 1)]
                # ---- detector row-chunk m ----
                for t in range(NBLK):
                    ps = psd.tile([128, 1024], F32, tag="psD")
                    lo = 128 * m if t == 0 else 0
                    for h in range(2):
                        a, b = max(lo, 512 * h), 512 * (h + 1)
                        if a >= b:
                            continue
                        nc.tensor.matmul(ps[:, a:b], lhs, t_c8[t][:, a:b],
                                         start=True, stop=True)
                    if t == 0:
                        nc.tensor.matmul(ps[:, 128 * m:128 * (m + 1)],
                                         t_nb[:, :], t_id[:, :],
                                         start=False, stop=True,
                                         skip_group_check=True)
                    slot = NBLK * m + t
                    if slot in ACT_SLOTS:
                        dum = dp.tile([128, 1024], BF16, tag="dum")
                        nc.scalar.activation(dum[:, lo:1024], ps[:, lo:1024],
                                             AF.Relu, bias=t_nT[:, m:m + 1],
                                             scale=1.0,
                                             accum_out=t_vm[:, slot:slot + 1])
                    else:
                        nc.vector.tensor_reduce(t_mg[:, slot:slot + 1],
                                                ps[:, lo:1024],
                                                axis=mybir.AxisListType.X,
                                                op=mybir.AluOpType.max)
                # ---- intra feature tiles scheduled for this m ----
                for _ in range(INTRA_SCHED[m]):
                    k = kin
                    kin += 1
                    ft = t_fs[k]
                    sqd = sqp.tile([128, 1024], F32, tag="sqd")
                    for h in range(2):
                        psq = psa.tile([128, 512], F32, tag="psA")
                        nc.tensor.matmul(psq[:], t_m[:, :],
                                         ft[:, 512 * h:512 * (h + 1)],
                                         start=True, stop=True)
                        nc.scalar.activation(sqd[:, 512 * h:512 * (h + 1)],
                                             psq[:], AF.Square)
                    nc.vector.tensor_reduce(
                        t_d2[:, 8 * k:8 * (k + 1)],
                        sqd[:].rearrange("p (t d) -> p t d", d=128),
                        axis=mybir.AxisListType.X, op=mybir.AluOpType.add)
                if m == 6:
                    # first 6 row-chunks' detector slots export under m=7
                    nc.sync.dma_start(vm[:, 0:NBLK * 6], t_vm[:, 0:NBLK * 6])
                    nc.sync.dma_start(mg[:, 0:NBLK * 6], t_mg[:, 0:NBLK * 6])
                    # intra done: hinge tail + ipart export hide under m=7
                    t_d = sp.tile([128, 128], F32, tag="d")
                    nc.scalar.activation(t_d[:], t_d2[:], AF.Sqrt)
                    t_w = sp.tile([128, 128], F32, tag="w")
                    nc.vector.tensor_scalar(t_w[:], t_d[:], MARGIN_INTRA, 0.0,
                                            op0=mybir.AluOpType.subtract,
                                            op1=mybir.AluOpType.max)
                    t_w2 = sp.tile([128, 128], F32, tag="w2")
                    t_acc = sp.tile([128, 1], F32, tag="acc")
                    nc.scalar.activation(t_w2[:], t_w[:], AF.Square,
                                         accum_out=t_acc[:])
                    nc.gpsimd.dma_start(ipart[:], t_acc[:])

            nc.scalar.dma_start(vm[:, NBLK * 6:], t_vm[:, NBLK * 6:])
            nc.sync.dma_start(mg[:, NBLK * 6:], t_mg[:, NBLK * 6:])
    nc.compile()
    return nc


def _get(name, builder):
    if name not in _cache:
        _cache[name] = builder()
    return _cache[name]


def _exact_inter_host(centers):
    """Exact fp64 inter loss (full fallback)."""
    c = centers.astype(np.float64)
    sq = (c * c).sum(1)
    tot = 0.0
    for i0 in range(0, G, 1024):
        blk = sq[i0:i0 + 1024, None] + sq[None, :] - 2.0 * (c[i0:i0 + 1024] @ c.T)
        d = np.sqrt(np.maximum(blk, 0.0))
        h = np.maximum(MARGIN_INTER - d, 0.0) ** 2
        iu = np.triu(np.ones((1024, G), dtype=bool), k=1 + i0)
        tot += h[iu].sum()
    return np.float32(tot / (G * (G - 1) / 2.0))


def kernel(path_fea):
    fea = np.ascontiguousarray(
        np.asarray(path_fea, dtype=np.float32).reshape(B, D))

    trace = bool(int(__import__("os").environ.get("KERNEL_TRACE", "0")))
    runkw = {}
    if trace:
        try:
            import trace_shim
            trace_shim.install()
            runkw = dict(trace=True)
        except ImportError:
            trace = False

    # ---------------- host glue: centers, fp8 layout, thresholds --------
    centers = fea.reshape(G, P, D).mean(axis=1)              # [G, D] f32
    sq = (centers.astype(np.float64) ** 2).sum(1)
    minsq = sq.min()
    c8 = centers.astype(ml_dtypes.float8_e4m3fn)
    c8f = c8.astype(np.float64)
    delta = centers.astype(np.float64) - c8f
    dn = np.sqrt((delta ** 2).sum(1))                        # ||delta_i||
    cn = np.maximum(np.sqrt(sq), np.sqrt((c8f ** 2).sum(1)))
    # rigorous per-row bound on |gram - fp8 gram| (+ f32 accumulation slack)
    eg = dn * cn.max() + dn.max() * cn + 0.01
    T = ((sq + minsq - MARGIN_INTER - 2.0 * eg) / 2.0).astype(np.float32)

    Pm = np.zeros((128, 128), np.float32)
    for s in range(128):
        Pm[s, (s // 16) * 16:(s // 16 + 1) * 16] = 1.0 / 16.0
    m128 = np.eye(128, dtype=np.float32) - Pm

    nbig = (-BIG * np.eye(128)).astype(ml_dtypes.float8_e4m3fn)
    idI = np.eye(128, dtype=np.float32).astype(ml_dtypes.float8_e4m3fn)

    ins = []
    for c in range(N_CORES):
        idx = (np.arange(NC2) + GL * c) % G
        ctr8c = np.ascontiguousarray(c8[idx].T)              # [128, NC2]
        negT = np.ascontiguousarray(
            -T[GL * c:GL * (c + 1)].reshape(8, 128).T)       # [128, 8]
        ins.append({"fea": fea[SL * c:SL * (c + 1)], "m128": m128,
                    "ctr8": ctr8c, "negT": negT, "nbig": nbig, "idI": idI})

    ncf = _get("fused", _build_fused)
    r = run_bass_kernel_spmd(ncf, ins, core_ids=list(range(N_CORES)), **runkw)
    if trace and r.exec_time_ns is not None:
        print(f"[fused] HW exec time: {r.exec_time_ns} ns")
        _last_traces["fused"] = r

    # ---------------- host reduction + certification ----------------
    ipart_sum = 0.0
    suspects = []
    finite = np.isfinite(T).all()
    for c in range(N_CORES):
        ipart_sum += float(r.results[c]["ipart"].astype(np.float64).sum())
        vmc = r.results[c]["vm"]
        mgc = r.results[c]["mg"]
        if not (finite and np.isfinite(vmc).all() and np.isfinite(mgc).all()):
            suspects.extend(range(GL * c, GL * (c + 1)))
            continue
        Tc = T[GL * c:GL * (c + 1)].reshape(8, 128).T        # [128, 8]
        for m in range(8):
            slots_a = [NBLK * m + t for t in range(NBLK)
                       if NBLK * m + t in ACT_SLOTS]
            slots_d = [NBLK * m + t for t in range(NBLK)
                       if NBLK * m + t not in ACT_SLOTS]
            bad = np.zeros(128, bool)
            if slots_a:
                bad |= (vmc[:, slots_a].sum(axis=1) > 0.0)
            if slots_d:
                bad |= (mgc[:, slots_d].max(axis=1) > Tc[:, m])
            for p in np.nonzero(bad)[0]:
                suspects.append(GL * c + 128 * m + int(p))
    intra = np.float32(ipart_sum / B)

    inter = np.float32(0.0)
    if suspects:
        # exact fp64 recheck of suspect rows against all centers
        cd = centers.astype(np.float64)
        sqd_ = (cd * cd).sum(1)
        ok = True
        for i in suspects:
            d2 = sqd_[i] + sqd_ - 2.0 * (cd @ cd[i])
            d2[i] = np.inf
            if d2.min() <= MARGIN_INTER ** 2:
                ok = False
                break
        if not ok:
            inter = _exact_inter_host(centers)
    return (inter, intra)


# revision 29
# speedup vs baseline: 1.1611x; 1.0825x over previous
"""LDA loss (inter/intra hinge) on 8 Trainium2 NeuronCores — fused launch.

Per core (data-parallel over B, hence over G = B/16 centers):

  Host glue: centers c = group means (O(G*D) numpy), sq = ||c||^2, fp8
    (e4m3) quantized centers rotated so each core's own 1024-center
    block leads, followed by the next 4 blocks (this symmetric schedule
    covers every unordered pair at least once).  Per-row certification
    thresholds T_i = (sq_i + min_sq - 1 - 2*eg_i)/2 where eg_i is a
    rigorous bound on the fp8 gram error derived from the exact
    quantization residuals.

  Device, one launch:
    - inter detector: psum = C_loc^T C_cols via fp8 matmuls; the own
      (diagonal) block only computes columns j >= 128*m (upper
      triangle), with -384*I added on the diagonal itself.  Each psum
      tile is consumed either by ACT relu(psum - T_i) with accumulation
      (violation mass, exactly 0 when all grams are certified-far) or by
      DVE max-reduce (per-row max gram); the split is load-balanced.
    - intra: diff = (I - P) @ x via f32r matmuls (P = group-mean
      selector); DVE bn_stats on the PSUM diff gives per-sample
      sum(diff^2) = s2 + s5 + 64*(s1^2 + s4^2); an exact hinge tail
      computes sum(max(d - 0.1, 0)^2) -> ipart.  Feature chunks are
      scheduled late (after their DMA lands) so the in-order PE never
      stalls the detector stream.

  Host check: rows with relu mass > 0 or max-gram above T_i are
    "suspect"; suspect rows are re-verified exactly in fp64 (never taken
    for in-margin data).  If every pair is certified > margin, every
    hinge term is exactly 0 and inter == 0.0 bit-equal to the reference;
    otherwise fall back to the exact fp64 computation.

  intra = sum(ipart) / B.
"""
import sys

if "/opt/trn_rl_repo" not in sys.path:
    sys.path.insert(0, "/opt/trn_rl_repo")

import numpy as np
import ml_dtypes

import concourse.bacc as bacc
import concourse.tile as tile
from concourse import mybir
from concourse.bass_utils import run_bass_kernel_spmd

N_CORES = 8
B, D, P = 131072, 128, 16
G = B // P                 # 8192 centers
GL = G // N_CORES          # 1024 local centers
SL = B // N_CORES          # 16384 local samples
NBLK = 5                   # column blocks per core in the detector
NC2 = NBLK * GL            # 5120 detector columns
NSLOT = NBLK * 8           # detector (m, t) slots
BIG = 384.0                # e4m3-exact diagonal suppressor
MARGIN_INTRA = 0.1
MARGIN_INTER = 1.0
# intra feature tiles processed in detector row-chunk m (DMA-arrival paced)
INTRA_SCHED = [0, 0, 2, 3, 3, 4, 4, 0]

F32 = mybir.dt.float32
F32R = mybir.dt.float32r
BF16 = mybir.dt.bfloat16
FP8 = mybir.dt.float8e4
AF = mybir.ActivationFunctionType

_cache = {}
_last_traces = {}


def _det_plan():
    """Static (m, t) -> engine assignment, load-balanced greedily.
    Returns set of slots handled by ACT (rest go to DVE max-reduce)."""
    act_slots = set()
    load_a = 23500.0   # ACT: 32 intra square evictions + hinge tail + accum reads
    load_d = 22000.0   # DVE: 32 intra reduces + memsets
    for m in range(8):
        for t in range(NBLK):
            w = 1024 - 128 * m if t == 0 else 1024
            ca = 356 + w * 0.833 + 286    # measured ACT relu+accum cost
            cd = 80 + w * 1.042 + 45      # measured DVE max-reduce cost
            if load_a + ca <= load_d + cd:
                act_slots.add(NBLK * m + t)
                load_a += ca
            else:
                load_d += cd
    return act_slots


ACT_SLOTS = _det_plan()


def _build_fused():
    nc = bacc.Bacc("TRN2", target_bir_lowering=False, debug=False,
                   num_devices=N_CORES)
    fea = nc.dram_tensor("fea", [SL, D], F32R, kind="ExternalInput").ap()
    m128 = nc.dram_tensor("m128", [128, 128], F32R, kind="ExternalInput").ap()
    ctr8 = nc.dram_tensor("ctr8", [128, NC2], FP8, kind="ExternalInput").ap()
    negT = nc.dram_tensor("negT", [128, 8], F32, kind="ExternalInput").ap()
    nbig = nc.dram_tensor("nbig", [128, 128], FP8, kind="ExternalInput").ap()
    idI = nc.dram_tensor("idI", [128, 128], FP8, kind="ExternalInput").ap()
    outp = nc.dram_tensor("outp", [128, 2 * NSLOT + 1], F32,
                          kind="ExternalOutput").ap()

    fea3 = fea.rearrange("(b p) d -> p b d", p=128)  # [128, 128, 128]

    with tile.TileContext(nc) as tc:
        with (
            tc.tile_pool(name="fea", bufs=1) as fp,
            tc.tile_pool(name="small", bufs=1) as sp,
            tc.tile_pool(name="dum", bufs=2) as dp,
            tc.tile_pool(name="sqd", bufs=2) as sqp,
            tc.tile_pool(name="psD", bufs=3, space="PSUM") as psd,
            tc.tile_pool(name="psA", bufs=2, space="PSUM") as psa,
        ):
            # detector columns first: PE starts as soon as block 0 lands
            t_c8 = []
            for k in range(NBLK):
                t = sp.tile([128, 1024], FP8, tag=f"c8{k}")
                eng = nc.scalar if k < 2 else nc.sync
                eng.dma_start(t[:], ctr8[:, 1024 * k:1024 * (k + 1)])
                t_c8.append(t)
            t_nT = sp.tile([128, 8], F32, tag="nT")
            nc.sync.dma_start(t_nT[:], negT[:])
            t_nb = sp.tile([128, 128], FP8, tag="nb")
            nc.sync.dma_start(t_nb[:], nbig[:])
            t_id = sp.tile([128, 128], FP8, tag="id")
            nc.sync.dma_start(t_id[:], idI[:])
            t_m = sp.tile([128, 128], F32R, tag="m")
            nc.sync.dma_start(t_m[:], m128[:])
            t_fs = []
            for k in range(16):
                t = fp.tile([128, 1024], F32R, tag=f"fea{k}")
                nc.sync.dma_start(
                    t[:].rearrange("p (b d) -> p b d", d=128),
                    fea3[:, 8 * k:8 * (k + 1), :])
                t_fs.append(t)
            # pull the sqrt_and_others ACT table load off the critical tail
            t_tb = sp.tile([128, 1], F32, tag="tb")
            nc.scalar.activation(t_tb[:], t_nT[:, 0:1], AF.Sqrt,
                                 bias=0.0, scale=0.0)

            t_out = sp.tile([128, 2 * NSLOT + 1], F32, tag="out")
            t_vm = t_out[:, 0:NSLOT]
            t_mg = t_out[:, NSLOT:2 * NSLOT]
            t_acc = t_out[:, 2 * NSLOT:2 * NSLOT + 1]
            nc.vector.memset(t_out[:], 0.0)
            t_d2 = sp.tile([128, 128], F32, tag="d2")

            kin = 0   # next intra feature tile
            for m in range(8):
                lhs = t_c8[0][:, 128 * m:128 * (m + 1)]
                # ---- detector row-chunk m ----
                for t in range(NBLK):
                    ps = psd.tile([128, 1024], F32, tag="psD")
                    lo = 128 * m if t == 0 else 0
                    for h in range(2):
                        a, b = max(lo, 512 * h), 512 * (h + 1)
                        if a >= b:
                            continue
                        nc.tensor.matmul(ps[:, a:b], lhs, t_c8[t][:, a:b],
                                         start=True, stop=True)
                    if t == 0:
                        nc.tensor.matmul(ps[:, 128 * m:128 * (m + 1)],
                                         t_nb[:, :], t_id[:, :],
                                         start=False, stop=True,
                                         skip_group_check=True)
                    slot = NBLK * m + t
                    if slot in ACT_SLOTS:
                        dum = dp.tile([128, 1024], BF16, tag="dum")
                        nc.scalar.activation(dum[:, lo:1024], ps[:, lo:1024],
                                             AF.Relu, bias=t_nT[:, m:m + 1],
                                             scale=1.0,
                                             accum_out=t_vm[:, slot:slot + 1])
                    else:
                        nc.vector.tensor_reduce(t_mg[:, slot:slot + 1],
                                                ps[:, lo:1024],
                                                axis=mybir.AxisListType.X,
                                                op=mybir.AluOpType.max)
                # ---- intra feature tiles scheduled for this m ----
                for _ in range(INTRA_SCHED[m]):
                    k = kin
                    kin += 1
                    ft = t_fs[k]
                    sqd = sqp.tile([128, 1024], F32, tag="sqd")
                    for h in range(2):
                        psq = psa.tile([128, 512], F32, tag="psA")
                        nc.tensor.matmul(psq[:], t_m[:, :],
                                         ft[:, 512 * h:512 * (h + 1)],
                                         start=True, stop=True)
                        nc.scalar.activation(sqd[:, 512 * h:512 * (h + 1)],
                                             psq[:], AF.Square)
                    nc.vector.tensor_reduce(
                        t_d2[:, 8 * k:8 * (k + 1)],
                        sqd[:].rearrange("p (t d) -> p t d", d=128),
                        axis=mybir.AxisListType.X, op=mybir.AluOpType.add)
                if m == 6:
                    # first 6 row-chunks' detector slots export under m=7
                    nc.sync.dma_start(outp[:, 0:NBLK * 6], t_vm[:, 0:NBLK * 6])
                    nc.sync.dma_start(outp[:, NSLOT:NSLOT + NBLK * 6],
                                      t_mg[:, 0:NBLK * 6])
                    # intra done: hinge tail hides under m=7
                    t_d = sp.tile([128, 128], F32, tag="d")
                    nc.scalar.activation(t_d[:], t_d2[:], AF.Sqrt)
                    t_w = sp.tile([128, 128], F32, tag="w")
                    nc.vector.tensor_scalar(t_w[:], t_d[:], MARGIN_INTRA, 0.0,
                                            op0=mybir.AluOpType.subtract,
                                            op1=mybir.AluOpType.max)
                    t_w2 = sp.tile([128, 128], F32, tag="w2")
                    nc.scalar.activation(t_w2[:], t_w[:], AF.Square,
                                         accum_out=t_acc)

            # one final export: last chunks of vm, mg tail, and ipart
            nc.sync.dma_start(outp[:, NBLK * 6:NSLOT], t_vm[:, NBLK * 6:])
            nc.scalar.dma_start(outp[:, NSLOT + NBLK * 6:],
                                t_out[:, NSLOT + NBLK * 6:])
    nc.compile()
    return nc


def _get(name, builder):
    if name not in _cache:
        _cache[name] = builder()
    return _cache[name]


def _exact_inter_host(centers):
    """Exact fp64 inter loss (full fallback)."""
    c = centers.astype(np.float64)
    sq = (c * c).sum(1)
    tot = 0.0
    for i0 in range(0, G, 1024):
        blk = sq[i0:i0 + 1024, None] + sq[None, :] - 2.0 * (c[i0:i0 + 1024] @ c.T)
        d = np.sqrt(np.maximum(blk, 0.0))
        h = np.maximum(MARGIN_INTER - d, 0.0) ** 2
        iu = np.triu(np.ones((1024, G), dtype=bool), k=1 + i0)
        tot += h[iu].sum()
    return np.float32(tot / (G * (G - 1) / 2.0))


def kernel(path_fea):
    fea = np.ascontiguousarray(
        np.asarray(path_fea, dtype=np.float32).reshape(B, D))

    trace = bool(int(__import__("os").environ.get("KERNEL_TRACE", "0")))
    runkw = {}
    if trace:
        try:
            import trace_shim
            trace_shim.install()
            runkw = dict(trace=True)
        except ImportError:
            trace = False

    # ---------------- host glue: centers, fp8 layout, thresholds --------
    centers = fea.reshape(G, P, D).mean(axis=1)              # [G, D] f32
    sq = (centers.astype(np.float64) ** 2).sum(1)
    minsq = sq.min()
    c8 = centers.astype(ml_dtypes.float8_e4m3fn)
    c8f = c8.astype(np.float64)
    delta = centers.astype(np.float64) - c8f
    dn = np.sqrt((delta ** 2).sum(1))                        # ||delta_i||
    cn = np.maximum(np.sqrt(sq), np.sqrt((c8f ** 2).sum(1)))
    # rigorous per-row bound on |gram - fp8 gram| (+ f32 accumulation slack)
    eg = dn * cn.max() + dn.max() * cn + 0.01
    T = ((sq + minsq - MARGIN_INTER - 2.0 * eg) / 2.0).astype(np.float32)

    Pm = np.zeros((128, 128), np.float32)
    for s in range(128):
        Pm[s, (s // 16) * 16:(s // 16 + 1) * 16] = 1.0 / 16.0
    m128 = np.eye(128, dtype=np.float32) - Pm

    nbig = (-BIG * np.eye(128)).astype(ml_dtypes.float8_e4m3fn)
    idI = np.eye(128, dtype=np.float32).astype(ml_dtypes.float8_e4m3fn)

    ins = []
    for c in range(N_CORES):
        idx = (np.arange(NC2) + GL * c) % G
        ctr8c = np.ascontiguousarray(c8[idx].T)              # [128, NC2]
        negT = np.ascontiguousarray(
            -T[GL * c:GL * (c + 1)].reshape(8, 128).T)       # [128, 8]
        ins.append({"fea": fea[SL * c:SL * (c + 1)], "m128": m128,
                    "ctr8": ctr8c, "negT": negT, "nbig": nbig, "idI": idI})

    ncf = _get("fused", _build_fused)
    r = run_bass_kernel_spmd(ncf, ins, core_ids=list(range(N_CORES)), **runkw)
    if trace and r.exec_time_ns is not None:
        print(f"[fused] HW exec time: {r.exec_time_ns} ns")
        _last_traces["fused"] = r

    # ---------------- host reduction + certification ----------------
    ipart_sum = 0.0
    suspects = []
    finite = np.isfinite(T).all()
    for c in range(N_CORES):
        outc = r.results[c]["outp"]
        ipart_sum += float(outc[:, 2 * NSLOT].astype(np.float64).sum())
        vmc = outc[:, 0:NSLOT]
        mgc = outc[:, NSLOT:2 * NSLOT]
        if not (finite and np.isfinite(vmc).all() and np.isfinite(mgc).all()):
            suspects.extend(range(GL * c, GL * (c + 1)))
            continue
        Tc = T[GL * c:GL * (c + 1)].reshape(8, 128).T        # [128, 8]
        for m in range(8):
            slots_a = [NBLK * m + t for t in range(NBLK)
                       if NBLK * m + t in ACT_SLOTS]
            slots_d = [NBLK * m + t for t in range(NBLK)
                       if NBLK * m + t not in ACT_SLOTS]
            bad = np.zeros(128, bool)
            if slots_a:
                bad |= (vmc[:, slots_a].sum(axis=1) > 0.0)
            if slots_d:
                bad |= (mgc[:, slots_d].max(axis=1) > Tc[:, m])
            for p in np.nonzero(bad)[0]:
                suspects.append(GL * c + 128 * m + int(p))
    intra = np.float32(ipart_sum / B)

    inter = np.float32(0.0)
    if suspects:
        # exact fp64 recheck of suspect rows against all centers
        cd = centers.astype(np.float64)
        sqd_ = (cd * cd).sum(1)
        ok = True
        for i in suspects:
            d2 = sqd_[i] + sqd_ - 2.0 * (cd @ cd[i])
            d2[i] = np.inf
            if d2.min() <= MARGIN_INTER ** 2:
                ok = False
                break
        if not ok:
            inter = _exact_inter_host(centers)
    return (inter, intra)
